# revision 29
# baseline (speedup 1.0000x reference)
"""Distributed multi-head attention for Trainium2 (8 NeuronCores).

Problem: B=2, S=2048, D=2048, H=16 heads, head_dim=128.
    out = softmax((x Wq^T)(x Wk^T)^T / sqrt(d)) (x Wv^T) Wo^T
(mask is all zeros, rotary_emb unused - both ignored.)

Sharding (Megatron-style tensor parallelism on heads): core c owns heads
{2c, 2c+1}; it runs q/k/v projections + attention for those heads over
both batch elements, producing attention output TRANSPOSED
([head_dim, seq]) per head.  A per-local-head 8-core AllToAll
redistributes from head-sharded to row-sharded form; each core then
applies the output projection to its 512-row slice of the flattened
(B*S) output.

v2 rewrite vs the bf16 baseline (574us):
 - fp16 everywhere (same PE speed as bf16, 8x less rounding error; the
   error budget is spent on speed-neutral simplifications instead).
 - softmax: scores for a chunk-PAIR land in one 2-bank PSUM tile
   [128,1024]; ONE Exp activation per pair halves ScalarE instruction
   overhead.  Denominators: DVE accumulates the sum of exp tiles, a
   gpsimd partition_all_reduce yields an already-broadcast [128,1024]
   sum (no separate broadcast step), DVE fast-reciprocal + multiply
   normalize while staging to the AllToAll buffer.
 - attention ordered h-major (b0h0, b1h0, A2A#0, b0h1, b1h1, A2A#1) so
   both AllToAlls overlap compute.
 - all large DMAs are single big-tile transfers (~70 issues vs ~290).
 - PSUM->SBUF projection copies run on the otherwise-idle ScalarE
   (phase-disjoint from the Exp work).
 - output projection keeps partials resident in PSUM across the two
   head passes (no f16 staging round-trip).

PSUM budget (8 banks x [128, 512] f32): tag "A" = 2 x [128,1024]
(4 banks; score pairs, then outproj partials), tag "B" = 4 x [128,512]
(4 banks; projection psums, then attn-V accumulators).
"""

import sys
import numpy as np

sys.path.insert(0, "/opt/trn_rl_repo")

B = 2
S = 2048
D = 2048
H = 16
HD = 128           # head dim
P = 128            # partitions
NCORES = 8
HPC = 2            # heads per core
KT = D // P        # 16 k-tiles of the contraction dim
KH = KT // 2       # k-tiles per half-group
NC = 4             # 512-wide token chunks per 2048
CH = 512           # chunk width
MS = B * S // NCORES  # per-core output row slice = 512
INV_SQRT_HD = float(1.0 / np.sqrt(HD))
EXP_BIAS = -1.3862943611198906   # -ln(4): keeps f16 exp values in range

_CACHE = {}


def _build():
    import concourse.tile as tile
    import concourse.bass_isa as bass_isa
    from concourse import bacc, mybir
    from contextlib import ExitStack

    dt = mybir.dt
    f16 = dt.float16
    f32 = dt.float32
    nc = bacc.Bacc("TRN2", target_bir_lowering=False, debug=False,
                   enable_asserts=False, num_devices=NCORES)

    # DRAM inputs (host-prepped layouts; see _prep_inputs)
    xg = nc.dram_tensor("xg", [B, NC, 2, P, KH * CH], f16,
                        kind="ExternalInput").ap()
    wq = nc.dram_tensor("wq", [P, KT * HPC * HD], f16, kind="ExternalInput").ap()
    wk = nc.dram_tensor("wk", [P, KT * HPC * HD], f16, kind="ExternalInput").ap()
    wv = nc.dram_tensor("wv", [P, KT * HPC * HD], f16, kind="ExternalInput").ap()
    wo = nc.dram_tensor("wo", [HPC, NC, P, NCORES * CH], f16,
                        kind="ExternalInput").ap()
    out = nc.dram_tensor("out", [MS, D], f32, kind="ExternalOutput").ap()

    rg = [list(range(NCORES))]

    with tile.TileContext(nc) as tc, ExitStack() as ctx:
        dram = ctx.enter_context(tc.tile_pool(name="dram", bufs=1, space="DRAM"))
        a2a_in = [dram.tile([NCORES * P, CH], f16, name=f"a2a_in{h}",
                            tag=f"a2a_in{h}") for h in range(HPC)]
        a2a_out = [dram.tile([NCORES * P, CH], f16, name=f"a2a_out{h}",
                             tag=f"a2a_out{h}") for h in range(HPC)]
        # h1 uses two half-width collectives (token halves) so output
        # projection pass 2 on mt{0,1} can start while the second half is
        # still in flight
        a2a1_in = [dram.tile([NCORES * P, CH // 2], f16, name=f"a2a1i{j}",
                             tag=f"a2a1i{j}") for j in range(2)]
        a2a1_out = [dram.tile([NCORES * P, CH // 2], f16, name=f"a2a1o{j}",
                              tag=f"a2a1o{j}") for j in range(2)]

        psum = ctx.enter_context(tc.tile_pool(name="psum", bufs=1, space="PSUM"))
        sb = ctx.enter_context(tc.tile_pool(name="sb", bufs=1))

        def psA(name):
            return psum.tile([P, 2 * CH], f32, tag="A", bufs=2, name=name)

        def psB(name):
            return psum.tile([P, CH], f32, tag="B", bufs=4, name=name)

        ebias = sb.tile([P, 1], f32, name="ebias", tag="ebias")
        nc.vector.memset(ebias[:], EXP_BIAS)

        # resident qkv weights, one big tile each (free idx = k*256 + j)
        wq_sb = sb.tile([P, KT * HPC * HD], f16, name="wq", tag="wq")
        wk_sb = sb.tile([P, KT * HPC * HD], f16, name="wk", tag="wk")
        wv_sb = sb.tile([P, KT * HPC * HD], f16, name="wv", tag="wv")

        qT_sb = [[None] * HPC for _ in range(B)]
        kT_sb = [[None] * HPC for _ in range(B)]
        v_sb = [[None] * KT for _ in range(B)]

        def load_x(b, c, eng0=None, eng1=None):
            """Two half-group DMAs for token chunk c of batch b."""
            t0 = sb.tile([P, KH * CH], f16, name=f"x{b}{c}0", tag="xg", bufs=7)
            t1 = sb.tile([P, KH * CH], f16, name=f"x{b}{c}1", tag="xg", bufs=7)
            (eng0 or nc.sync).dma_start(t0[:], xg[b, c, 0])
            (eng1 or nc.gpsimd).dma_start(t1[:], xg[b, c, 1])
            return (t0, t1)

        def xsl(xt, k, lo, w):
            """[P, w] slice of x for k-tile k, token offset lo in its chunk."""
            return xt[k // KH][:, (k % KH) * CH + lo:(k % KH) * CH + lo + w]

        def proj_b(b, xts):
            # chunk-pair-major: q, k, then v for a chunk pair, then the next
            # pair - frees x chunks as early as possible (the b1 prefetch
            # rotates through the same buffers).  q/k are weight-stationary
            # over the pair (2 matmuls per LDWEIGHTS if walrus dedupes).
            for cp in range(0, NC, 2):
                for (w_sb, dst, nm) in ((wq_sb, qT_sb, "q"), (wk_sb, kT_sb, "k")):
                    for h in range(HPC):
                        if cp == 0:
                            dst[b][h] = sb.tile([P, S], f16, name=f"{nm}T{b}{h}",
                                                tag="qk", bufs=8)
                        dstt = dst[b][h]
                        pq0 = psB(f"p{nm}{b}{h}{cp}0")
                        pq1 = psB(f"p{nm}{b}{h}{cp}1")
                        for k in range(KT):
                            wsl = w_sb[:, k * HPC * HD + h * HD:
                                       k * HPC * HD + (h + 1) * HD]
                            nc.tensor.matmul(pq0[:], wsl,
                                             xsl(xts[cp], k, 0, CH),
                                             start=(k == 0), stop=(k == KT - 1))
                            nc.tensor.matmul(pq1[:], wsl,
                                             xsl(xts[cp + 1], k, 0, CH),
                                             start=(k == 0), stop=(k == KT - 1))
                        nc.scalar.copy(out=dstt[:, cp * CH:(cp + 1) * CH],
                                       in_=pq0[:])
                        nc.scalar.copy(out=dstt[:, (cp + 1) * CH:(cp + 2) * CH],
                                       in_=pq1[:])
                # v seq-tiles living in this chunk pair
                for st in range(4 * cp, 4 * cp + 8):
                    vt = sb.tile([P, HPC * HD], f16, name=f"v{b}{st}", tag="v",
                                 bufs=2 * KT + 2)
                    v_sb[b][st] = vt
                    pv = psB(f"pv{b}{st}")
                    c, lo = st // NC, (st % NC) * P
                    for k in range(KT):
                        nc.tensor.matmul(pv[:, :HPC * HD], xsl(xts[c], k, lo, P),
                                         wv_sb[:, k * HPC * HD:
                                               (k + 1) * HPC * HD],
                                         start=(k == 0), stop=(k == KT - 1))
                    nc.scalar.copy(out=vt[:], in_=pv[:, :HPC * HD])

        # deferred normalization: the gpsimd all-reduce is issued at chunk-pair
        # end; the DVE reciprocal+scale ops (which would block the in-order
        # DVE queue behind the reduce) are dribbled one-per-slot into the
        # NEXT chunk-pair's st-loop so the DVE never hiccups the exp pipe.
        pending = []   # list of closures, executed one per flush slot

        def stage_norm(pav, red, h, g0):
            stg = sb.tile([P, 2 * CH], f16, name=f"stg{h}{g0}", tag="stg",
                          bufs=2)

            def mk_recip(i):
                def op():
                    sl = slice(i * CH, (i + 1) * CH)
                    nc.vector.reciprocal_approx_fast(out=red[:, sl],
                                                     in_=red[:, sl])
                return op

            def mk_mult(i):
                def op():
                    sl = slice(i * CH, (i + 1) * CH)
                    nc.vector.tensor_tensor(out=stg[:, sl], in0=pav[i][:],
                                            in1=red[:, sl],
                                            op=mybir.AluOpType.mult)
                return op

            def send():
                stgv = stg[:].rearrange("p (g c) -> p g c", g=2)
                if h == 0:
                    dst = (a2a_in[0].rearrange("(g p) c -> g p c", p=P)
                           [g0:g0 + 2].transpose([1, 0, 2]))
                    nc.sync.dma_start(dst, stgv)
                else:
                    for j in range(2):   # token halves -> split collectives
                        dst = (a2a1_in[j].rearrange("(g p) c -> g p c", p=P)
                               [g0:g0 + 2].transpose([1, 0, 2]))
                        nc.sync.dma_start(
                            dst, stgv[:, :, j * (CH // 2):(j + 1) * (CH // 2)])

            pending.extend([mk_recip(0), mk_mult(0), mk_recip(1), mk_mult(1),
                            send])

        def flush_norm():
            while pending:
                pending.pop(0)()

        def attn_bh(b, h):
            qT, kT_, vs = qT_sb[b][h], kT_sb[b][h], v_sb[b]
            for cp in range(0, NC, 2):
                g0 = NC * b + cp      # a2a destination slice of chunk cp
                pav0 = psB(f"pav{b}{h}{cp}0")
                pav1 = psB(f"pav{b}{h}{cp}1")
                pav = (pav0, pav1)
                sacc = sb.tile([P, 2 * CH], f16, name=f"sa{b}{h}{cp}",
                               tag="sacc", bufs=2)
                ets = {}
                # LAG-1 software pipeline: scores(st) ahead of attnV(st-1)
                for st in range(KT + 1):
                    # early slots: the multiplies free the previous pair's
                    # PSUM accumulators well before anyone re-needs them
                    if st in (3, 4, 5, 6, 7) and pending:
                        pending.pop(0)()
                    if st < KT:
                        ps = psA(f"ps{b}{h}{cp}{st}")
                        kslice = kT_[:, st * P:(st + 1) * P]
                        for i in range(2):
                            nc.tensor.matmul(
                                ps[:, i * CH:(i + 1) * CH], kslice,
                                qT[:, (cp + i) * CH:(cp + i + 1) * CH],
                                start=True, stop=True)
                        et = sb.tile([P, 2 * CH], f16, name=f"e{b}{h}{cp}{st}",
                                     tag="exp", bufs=2)
                        nc.scalar.activation(et[:], ps[:],
                                             mybir.ActivationFunctionType.Exp,
                                             bias=ebias[:], scale=INV_SQRT_HD)
                        ets[st] = et
                        if st == 0:
                            nc.vector.tensor_copy(out=sacc[:], in_=et[:])
                        else:
                            nc.vector.tensor_tensor(out=sacc[:], in0=sacc[:],
                                                    in1=et[:],
                                                    op=mybir.AluOpType.add)
                    if st >= 1:
                        sv = st - 1
                        et = ets.pop(sv)
                        vsl = vs[sv][:, h * HD:(h + 1) * HD]
                        for i in range(2):
                            nc.tensor.matmul(pav[i][:], vsl,
                                             et[:, i * CH:(i + 1) * CH],
                                             start=(sv == 0), stop=(sv == KT - 1))
                # denominator all-reduce starts now (idle gpsimd), split per
                # chunk so the final flush's serial chain is short
                red = sb.tile([P, 2 * CH], f32, name=f"red{b}{h}{cp}", tag="red",
                              bufs=2)
                for i in range(2):
                    sl = slice(i * CH, (i + 1) * CH)
                    nc.gpsimd.partition_all_reduce(red[:, sl], sacc[:, sl], P,
                                                   bass_isa.ReduceOp.add)
                stage_norm(pav, red, h, NC * b + cp)

        # ---------------- schedule ----------------
        # batch-0 x streams on the sync+gpsimd rings while the weights ride
        # the scalar ring, so the first q chain's inputs arrive in parallel
        nc.scalar.dma_start(wq_sb[:], wq)
        xts = {}
        xts[(0, 0)] = load_x(0, 0)
        xts[(0, 1)] = load_x(0, 1)
        nc.scalar.dma_start(wk_sb[:], wk)
        xts[(0, 2)] = load_x(0, 2)
        nc.scalar.dma_start(wv_sb[:], wv)
        xts[(0, 3)] = load_x(0, 3)

        proj_b(0, [xts[(0, c)] for c in range(NC)])
        for c in range(NC):           # prefetch batch 1 during b0 h0 attention
            xts[(1, c)] = load_x(1, c)
        attn_bh(0, 0)
        flush_norm()                  # DVE stall here overlaps proj_b(1)
        proj_b(1, [xts[(1, c)] for c in range(NC)])
        attn_bh(1, 0)
        flush_norm()                  # a2a#0 needs the staged h0 chunks
        nc.gpsimd.collective_compute(
            "AllToAll", mybir.AluOpType.bypass, replica_groups=rg,
            ins=[a2a_in[0].opt()], outs=[a2a_out[0].opt()])

        # af/wo for the first outproj round arrive under the h1 attention
        wo_sb = [[None] * NC for _ in range(HPC)]
        for oc in range(2):
            for h in range(HPC):
                t = sb.tile([P, NCORES * CH], f16, name=f"wo{h}{oc}", tag="wo",
                            bufs=4)
                nc.sync.dma_start(t[:], wo[h, oc])
                wo_sb[h][oc] = t
        af = [None, None]
        af[0] = sb.tile([P, NCORES * CH], f16, name="af0", tag="af0")
        nc.sync.dma_start(af[0][:],
                          a2a_out[0].rearrange("(i p) c -> i p c", p=P)
                          .transpose([1, 0, 2]))

        attn_bh(0, 1)
        attn_bh(1, 1)
        flush_norm()                  # the a2a#1 halves need the staged chunks
        for j in range(2):
            nc.gpsimd.collective_compute(
                "AllToAll", mybir.AluOpType.bypass, replica_groups=rg,
                ins=[a2a1_in[j].opt()], outs=[a2a1_out[j].opt()])
        # round-2 wo loads ride the now-idle gpsimd queue; they
        # allocation-block until round 1 frees the buffers (harmless there,
        # and the transfers overlap round 1's tail)
        for h in range(HPC):
            for oc in (2, 3):
                t = sb.tile([P, NCORES * CH], f16, name=f"wo{h}{oc}", tag="wo",
                            bufs=4)
                nc.gpsimd.dma_start(t[:], wo[h, oc])
                wo_sb[h][oc] = t

        # h1 features as two token-half tiles with independent deps, so
        # pass 2 on mt{0,1} starts as soon as collective half 0 lands
        af1 = [sb.tile([P, NCORES * CH // 2], f16, name=f"af1{j}", tag=f"af1{j}")
               for j in range(2)]
        for j in range(2):
            nc.sync.dma_start(af1[j][:],
                              a2a1_out[j].rearrange("(i p) c -> i p c", p=P)
                              .transpose([1, 0, 2]))

        def aslf(h, i, mt):
            if h == 0:
                return af[0][:, i * CH + mt * P:i * CH + (mt + 1) * P]
            j, m = divmod(mt, 2)
            return af1[j][:, i * (CH // 2) + m * P:i * (CH // 2) + (m + 1) * P]

        # ---------------- output projection ----------------
        # oc-pair rounds; ALL FOUR mt partials for the pair stay resident in
        # PSUM (2 [128,1024] A-tiles for mt{0,1} + 4 [128,512] B-tiles for
        # mt{2,3}) across the h0/h1 passes, so the full h0 pass of a round
        # (64 matmuls) covers the AllToAll#1 latency before pass h1 needs it.
        outv = out.rearrange("(mt p) (oc c) -> mt p oc c", p=P, c=CH)
        for ocr in range(2):               # oc pair rounds: {0,1}, {2,3}
            ocs = (2 * ocr, 2 * ocr + 1)
            poA = {oc: psA(f"po{oc}") for oc in ocs}
            poB = {(oc, mt): psB(f"po{oc}{mt}") for oc in ocs for mt in (2, 3)}

            def po_slot(oc, mt):
                if mt < 2:
                    return poA[oc][:, mt * CH:(mt + 1) * CH]
                return poB[(oc, mt)][:]

            for h in range(HPC):
                for mt in range(4):
                    for i in range(NCORES):
                        asl = aslf(h, i, mt)
                        for oc in ocs:
                            nc.tensor.matmul(
                                po_slot(oc, mt), asl,
                                wo_sb[h][oc][:, i * CH:(i + 1) * CH],
                                start=(h == 0 and i == 0),
                                stop=(h == 1 and i == NCORES - 1))
            for oc in ocs:
                for mt in range(4):
                    ot = sb.tile([P, CH], f32, name=f"ot{oc}{mt}", tag="ot",
                                 bufs=2)
                    nc.scalar.copy(out=ot[:], in_=po_slot(oc, mt))
                    nc.sync.dma_start(outv[mt, :, oc], ot[:])

    nc.compile()
    return nc


def _prep_inputs(x, Wq, Wk, Wv, Wo):
    f16 = np.float16
    # x half-chunk groups [B, NC, 2, P, KH*CH]:
    # (b,c,hf,p, k'*CH+ch) = x[b, c*CH+ch, (hf*KH+k')*P+p]
    xt = np.ascontiguousarray(
        x.transpose(0, 2, 1).reshape(B, 2, KH, P, NC, CH)
        .transpose(0, 4, 1, 3, 2, 5).reshape(B, NC, 2, P, KH * CH)).astype(f16)

    def wshard(W, core):
        sl = slice(core * HPC * HD, (core + 1) * HPC * HD)
        return np.ascontiguousarray(
            W[sl].T.reshape(KT, P, HPC * HD).transpose(1, 0, 2)
            .reshape(P, KT * HPC * HD)).astype(f16)

    woh = np.ascontiguousarray(
        Wo.T.reshape(NCORES, HPC, P, NC, CH).transpose(1, 3, 2, 0, 4)
        .reshape(HPC, NC, P, NCORES * CH)).astype(f16)
    in_maps = []
    for core in range(NCORES):
        in_maps.append({
            "xg": xt,
            "wq": wshard(Wq, core),
            "wk": wshard(Wk, core),
            "wv": wshard(Wv, core),
            "wo": woh,
        })
    return in_maps


def kernel(x, rotary_emb, mask, Wq, Wk, Wv, Wo, _trace=False):
    x = np.asarray(x, dtype=np.float32)
    Wq = np.asarray(Wq, dtype=np.float32)
    Wk = np.asarray(Wk, dtype=np.float32)
    Wv = np.asarray(Wv, dtype=np.float32)
    Wo = np.asarray(Wo, dtype=np.float32)

    if "nc" not in _CACHE:
        _CACHE["nc"] = _build()
    nc = _CACHE["nc"]

    from concourse.bass_utils import run_bass_kernel_spmd
    in_maps = _prep_inputs(x, Wq, Wk, Wv, Wo)
    res = run_bass_kernel_spmd(nc, in_maps, core_ids=list(range(NCORES)),
                               trace=_trace)
    _CACHE["last_result"] = res

    flat = np.empty((B * S, D), dtype=np.float32)
    for core in range(NCORES):
        flat[core * MS:(core + 1) * MS, :] = res.results[core]["out"]
    return flat.reshape(B, S, D)


# revision 39
# speedup vs baseline: 1.1641x; 1.1641x over previous
"""Distributed multi-head attention for Trainium2 (8 NeuronCores).

Problem: B=2, S=2048, D=2048, H=16 heads, head_dim=128.
    out = softmax((x Wq^T)(x Wk^T)^T / sqrt(d)) (x Wv^T) Wo^T
(mask is all zeros, rotary_emb unused - both ignored.)

Sharding (Megatron-style tensor parallelism on heads): core c owns heads
{2c, 2c+1}; it runs q/k/v projections + attention for those heads over
both batch elements, producing attention output TRANSPOSED
([head_dim, seq]) per head.  A per-local-head 8-core AllToAll
redistributes from head-sharded to row-sharded form; each core then
applies the output projection to its 512-row slice of the flattened
(B*S) output.

v2 rewrite vs the bf16 baseline (574us):
 - fp16 everywhere (same PE speed as bf16, 8x less rounding error; the
   error budget is spent on speed-neutral simplifications instead).
 - softmax: scores for a chunk-PAIR land in one 2-bank PSUM tile
   [128,1024]; ONE Exp activation per pair halves ScalarE instruction
   overhead.  Denominators: DVE accumulates the sum of exp tiles, a
   gpsimd partition_all_reduce yields an already-broadcast [128,1024]
   sum (no separate broadcast step), DVE fast-reciprocal + multiply
   normalize while staging to the AllToAll buffer.
 - attention ordered h-major (b0h0, b1h0, A2A#0, b0h1, b1h1, A2A#1) so
   both AllToAlls overlap compute.
 - all large DMAs are single big-tile transfers (~70 issues vs ~290).
 - PSUM->SBUF projection copies run on the otherwise-idle ScalarE
   (phase-disjoint from the Exp work).
 - output projection keeps partials resident in PSUM across the two
   head passes (no f16 staging round-trip).

PSUM budget (8 banks x [128, 512] f32): tag "A" = 2 x [128,1024]
(4 banks; score pairs, then outproj partials), tag "B" = 4 x [128,512]
(4 banks; projection psums, then attn-V accumulators).
"""

import sys
import numpy as np

sys.path.insert(0, "/opt/trn_rl_repo")

B = 2
S = 2048
D = 2048
H = 16
HD = 128           # head dim
P = 128            # partitions
NCORES = 8
HPC = 2            # heads per core
KT = D // P        # 16 k-tiles of the contraction dim
KH = KT // 2       # k-tiles per half-group
NC = 4             # 512-wide token chunks per 2048
CH = 512           # chunk width
MS = B * S // NCORES  # per-core output row slice = 512
INV_SQRT_HD = float(1.0 / np.sqrt(HD))
EXP_BIAS = -1.3862943611198906   # -ln(4): keeps f16 exp values in range

_CACHE = {}


def _build():
    import concourse.tile as tile
    import concourse.bass_isa as bass_isa
    from concourse import bacc, mybir
    from contextlib import ExitStack

    dt = mybir.dt
    f16 = dt.float16
    f32 = dt.float32
    nc = bacc.Bacc("TRN2", target_bir_lowering=False, debug=False,
                   enable_asserts=False, num_devices=NCORES)

    # DRAM inputs (host-prepped layouts; see _prep_inputs)
    xg = nc.dram_tensor("xg", [B, NC, 2, P, KH * CH], f16,
                        kind="ExternalInput").ap()
    wq = nc.dram_tensor("wq", [P, KT * HPC * HD], f16, kind="ExternalInput").ap()
    wk = nc.dram_tensor("wk", [P, KT * HPC * HD], f16, kind="ExternalInput").ap()
    wv = nc.dram_tensor("wv", [P, KT * HPC * HD], f16, kind="ExternalInput").ap()
    wo = nc.dram_tensor("wo", [HPC, NC, P, NCORES * CH], f16,
                        kind="ExternalInput").ap()
    out = nc.dram_tensor("out", [MS, D], f32, kind="ExternalOutput").ap()

    rg = [list(range(NCORES))]

    with tile.TileContext(nc) as tc, ExitStack() as ctx:
        dram = ctx.enter_context(tc.tile_pool(name="dram", bufs=1, space="DRAM"))
        a2a_in = [dram.tile([NCORES * P, CH], f16, name=f"a2a_in{h}",
                            tag=f"a2a_in{h}") for h in range(HPC)]
        a2a_out = [dram.tile([NCORES * P, CH], f16, name=f"a2a_out{h}",
                             tag=f"a2a_out{h}") for h in range(HPC)]
        # h1 uses two half-width collectives (token halves) so output
        # projection pass 2 on mt{0,1} can start while the second half is
        # still in flight
        a2a1_in = [dram.tile([NCORES * P, CH // 2], f16, name=f"a2a1i{j}",
                             tag=f"a2a1i{j}") for j in range(2)]
        a2a1_out = [dram.tile([NCORES * P, CH // 2], f16, name=f"a2a1o{j}",
                              tag=f"a2a1o{j}") for j in range(2)]

        psum = ctx.enter_context(tc.tile_pool(name="psum", bufs=1, space="PSUM"))
        sb = ctx.enter_context(tc.tile_pool(name="sb", bufs=1))

        def psA(name):
            return psum.tile([P, 2 * CH], f32, tag="A", bufs=2, name=name)

        def psB(name):
            return psum.tile([P, CH], f32, tag="B", bufs=4, name=name)

        ebias = sb.tile([P, 1], f32, name="ebias", tag="ebias")
        nc.vector.memset(ebias[:], EXP_BIAS)
        ones = sb.tile([P, 1], f16, name="ones", tag="ones")
        nc.vector.memset(ones[:], 1.0)

        # resident qkv weights, one big tile each (free idx = k*256 + j)
        wq_sb = sb.tile([P, KT * HPC * HD], f16, name="wq", tag="wq")
        wk_sb = sb.tile([P, KT * HPC * HD], f16, name="wk", tag="wk")
        wv_sb = sb.tile([P, KT * HPC * HD], f16, name="wv", tag="wv")

        qT_sb = [[None] * HPC for _ in range(B)]
        kT_sb = [[None] * HPC for _ in range(B)]
        v_sb = [[None] * KT for _ in range(B)]

        def load_x(b, c, eng0=None, eng1=None):
            """Two half-group DMAs for token chunk c of batch b."""
            t0 = sb.tile([P, KH * CH], f16, name=f"x{b}{c}0", tag="xg", bufs=7)
            t1 = sb.tile([P, KH * CH], f16, name=f"x{b}{c}1", tag="xg", bufs=7)
            (eng0 or nc.sync).dma_start(t0[:], xg[b, c, 0])
            (eng1 or nc.gpsimd).dma_start(t1[:], xg[b, c, 1])
            return (t0, t1)

        def xsl(xt, k, lo, w):
            """[P, w] slice of x for k-tile k, token offset lo in its chunk."""
            return xt[k // KH][:, (k % KH) * CH + lo:(k % KH) * CH + lo + w]

        def proj_b(b, xts):
            # chunk-pair-major: q, k, then v for a chunk pair, then the next
            # pair - frees x chunks as early as possible (the b1 prefetch
            # rotates through the same buffers).  q/k are weight-stationary
            # over the pair (2 matmuls per LDWEIGHTS if walrus dedupes).
            for cp in range(0, NC, 2):
                for (w_sb, dst, nm) in ((wq_sb, qT_sb, "q"), (wk_sb, kT_sb, "k")):
                    for h in range(HPC):
                        if cp == 0:
                            dst[b][h] = sb.tile([P, S], f16, name=f"{nm}T{b}{h}",
                                                tag="qk", bufs=8)
                        dstt = dst[b][h]
                        pq0 = psB(f"p{nm}{b}{h}{cp}0")
                        pq1 = psB(f"p{nm}{b}{h}{cp}1")
                        for k in range(KT):
                            wsl = w_sb[:, k * HPC * HD + h * HD:
                                       k * HPC * HD + (h + 1) * HD]
                            nc.tensor.matmul(pq0[:], wsl,
                                             xsl(xts[cp], k, 0, CH),
                                             start=(k == 0), stop=(k == KT - 1))
                            nc.tensor.matmul(pq1[:], wsl,
                                             xsl(xts[cp + 1], k, 0, CH),
                                             start=(k == 0), stop=(k == KT - 1))
                        nc.scalar.copy(out=dstt[:, cp * CH:(cp + 1) * CH],
                                       in_=pq0[:])
                        nc.scalar.copy(out=dstt[:, (cp + 1) * CH:(cp + 2) * CH],
                                       in_=pq1[:])
                # v seq-tiles living in this chunk pair
                for st in range(4 * cp, 4 * cp + 8):
                    vt = sb.tile([P, HPC * HD], f16, name=f"v{b}{st}", tag="v",
                                 bufs=2 * KT + 2)
                    v_sb[b][st] = vt
                    pv = psB(f"pv{b}{st}")
                    c, lo = st // NC, (st % NC) * P
                    for k in range(KT):
                        nc.tensor.matmul(pv[:, :HPC * HD], xsl(xts[c], k, lo, P),
                                         wv_sb[:, k * HPC * HD:
                                               (k + 1) * HPC * HD],
                                         start=(k == 0), stop=(k == KT - 1))
                    nc.scalar.copy(out=vt[:], in_=pv[:, :HPC * HD])

        # deferred normalization: denominators come from a PE ones-matmul
        # (dn row in PSUM, ~0.4us instead of two serial ~4us gpsimd
        # all-reduces), then eager per-half DVE reciprocal + gpsimd
        # partition-broadcast.  Only the pav-consuming multiplies and the
        # staging DMA are dribbled into the next chunk-pair's first st
        # slots, so the previous pair's PSUM accumulators free early.
        pending = []   # list of closures, executed one per flush slot

        def stage_norm(pav, rb, h, g0):
            stg = sb.tile([P, 2 * CH], f16, name=f"stg{h}{g0}", tag="stg",
                          bufs=2)

            def mk_mult(i):
                def op():
                    sl = slice(i * CH, (i + 1) * CH)
                    nc.vector.tensor_tensor(out=stg[:, sl], in0=pav[i][:],
                                            in1=rb[:, sl],
                                            op=mybir.AluOpType.mult)
                return op

            def send():
                stgv = stg[:].rearrange("p (g c) -> p g c", g=2)
                if h == 0:
                    dst = (a2a_in[0].rearrange("(g p) c -> g p c", p=P)
                           [g0:g0 + 2].transpose([1, 0, 2]))
                    nc.sync.dma_start(dst, stgv)
                else:
                    for j in range(2):   # token halves -> split collectives
                        dst = (a2a1_in[j].rearrange("(g p) c -> g p c", p=P)
                               [g0:g0 + 2].transpose([1, 0, 2]))
                        nc.sync.dma_start(
                            dst, stgv[:, :, j * (CH // 2):(j + 1) * (CH // 2)])

            pending.extend([mk_mult(0), mk_mult(1), send])

        def flush_norm():
            while pending:
                pending.pop(0)()

        def attn_bh(b, h):
            qT, kT_, vs = qT_sb[b][h], kT_sb[b][h], v_sb[b]
            for cp in range(0, NC, 2):
                g0 = NC * b + cp      # a2a destination slice of chunk cp
                pav0 = psB(f"pav{b}{h}{cp}0")
                pav1 = psB(f"pav{b}{h}{cp}1")
                pav = (pav0, pav1)
                sacc = sb.tile([P, 2 * CH], f16, name=f"sa{b}{h}{cp}",
                               tag="sacc", bufs=2)
                ets = {}
                # LAG-1 software pipeline: scores(st) ahead of attnV(st-1)
                for st in range(KT + 1):
                    if st in (1, 2, 3) and pending:
                        pending.pop(0)()
                    if st < KT:
                        ps = psA(f"ps{b}{h}{cp}{st}")
                        kslice = kT_[:, st * P:(st + 1) * P]
                        for i in range(2):
                            nc.tensor.matmul(
                                ps[:, i * CH:(i + 1) * CH], kslice,
                                qT[:, (cp + i) * CH:(cp + i + 1) * CH],
                                start=True, stop=True)
                        et = sb.tile([P, 2 * CH], f16, name=f"e{b}{h}{cp}{st}",
                                     tag="exp", bufs=2)
                        nc.scalar.activation(et[:], ps[:],
                                             mybir.ActivationFunctionType.Exp,
                                             bias=ebias[:], scale=INV_SQRT_HD)
                        ets[st] = et
                        if st == 0:
                            nc.vector.tensor_copy(out=sacc[:], in_=et[:])
                        else:
                            nc.vector.tensor_tensor(out=sacc[:], in0=sacc[:],
                                                    in1=et[:],
                                                    op=mybir.AluOpType.add)
                    if st >= 1:
                        sv = st - 1
                        et = ets.pop(sv)
                        vsl = vs[sv][:, h * HD:(h + 1) * HD]
                        for i in range(2):
                            nc.tensor.matmul(pav[i][:], vsl,
                                             et[:, i * CH:(i + 1) * CH],
                                             start=(sv == 0), stop=(sv == KT - 1))
                # denominators: PE ones-matmul reduces sacc over partitions
                # into a PSUM row; per-half DVE reciprocal then gpsimd
                # partition-broadcast replicate it for the multiplies
                dn = psA(f"dn{b}{h}{cp}")
                redrow = sb.tile([1, 2 * CH], f32, name=f"rr{b}{h}{cp}",
                                 tag="redrow", bufs=1)
                rb = sb.tile([P, 2 * CH], f32, name=f"rb{b}{h}{cp}", tag="red",
                             bufs=2)
                for i in range(2):
                    sl = slice(i * CH, (i + 1) * CH)
                    nc.tensor.matmul(dn[:1, sl], ones[:], sacc[:, sl],
                                     start=True, stop=True)
                for i in range(2):
                    sl = slice(i * CH, (i + 1) * CH)
                    nc.vector.reciprocal_approx_fast(out=redrow[:, sl],
                                                     in_=dn[:1, sl])
                    nc.gpsimd.partition_broadcast(rb[:, sl], redrow[:, sl])
                stage_norm(pav, rb, h, NC * b + cp)

        # ---------------- schedule ----------------
        # batch-0 x streams on the sync+gpsimd rings while the weights ride
        # the scalar ring, so the first q chain's inputs arrive in parallel
        nc.scalar.dma_start(wq_sb[:], wq)
        xts = {}
        xts[(0, 0)] = load_x(0, 0)
        xts[(0, 1)] = load_x(0, 1)
        nc.scalar.dma_start(wk_sb[:], wk)
        xts[(0, 2)] = load_x(0, 2)
        nc.scalar.dma_start(wv_sb[:], wv)
        xts[(0, 3)] = load_x(0, 3)

        proj_b(0, [xts[(0, c)] for c in range(NC)])
        for c in range(NC):           # prefetch batch 1 during b0 h0 attention
            xts[(1, c)] = load_x(1, c)
        attn_bh(0, 0)
        flush_norm()                  # DVE stall here overlaps proj_b(1)
        proj_b(1, [xts[(1, c)] for c in range(NC)])
        attn_bh(1, 0)
        flush_norm()                  # a2a#0 needs the staged h0 chunks
        nc.gpsimd.collective_compute(
            "AllToAll", mybir.AluOpType.bypass, replica_groups=rg,
            ins=[a2a_in[0].opt()], outs=[a2a_out[0].opt()])

        # af/wo for the first outproj round arrive under the h1 attention
        wo_sb = [[None] * NC for _ in range(HPC)]
        for oc in range(2):
            for h in range(HPC):
                t = sb.tile([P, NCORES * CH], f16, name=f"wo{h}{oc}", tag="wo",
                            bufs=4)
                nc.sync.dma_start(t[:], wo[h, oc])
                wo_sb[h][oc] = t
        af = [None, None]
        af[0] = sb.tile([P, NCORES * CH], f16, name="af0", tag="af0")
        nc.sync.dma_start(af[0][:],
                          a2a_out[0].rearrange("(i p) c -> i p c", p=P)
                          .transpose([1, 0, 2]))

        attn_bh(0, 1)
        attn_bh(1, 1)
        flush_norm()                  # the a2a#1 halves need the staged chunks
        for j in range(2):
            nc.gpsimd.collective_compute(
                "AllToAll", mybir.AluOpType.bypass, replica_groups=rg,
                ins=[a2a1_in[j].opt()], outs=[a2a1_out[j].opt()])
        # round-2 wo loads ride the now-idle gpsimd queue; they
        # allocation-block until round 1 frees the buffers (harmless there,
        # and the transfers overlap round 1's tail)
        for h in range(HPC):
            for oc in (2, 3):
                t = sb.tile([P, NCORES * CH], f16, name=f"wo{h}{oc}", tag="wo",
                            bufs=4)
                nc.gpsimd.dma_start(t[:], wo[h, oc])
                wo_sb[h][oc] = t

        # h1 features as two token-half tiles with independent deps, so
        # pass 2 on mt{0,1} starts as soon as collective half 0 lands
        af1 = [sb.tile([P, NCORES * CH // 2], f16, name=f"af1{j}", tag=f"af1{j}")
               for j in range(2)]
        for j in range(2):
            nc.sync.dma_start(af1[j][:],
                              a2a1_out[j].rearrange("(i p) c -> i p c", p=P)
                              .transpose([1, 0, 2]))

        def aslf(h, i, mt):
            if h == 0:
                return af[0][:, i * CH + mt * P:i * CH + (mt + 1) * P]
            j, m = divmod(mt, 2)
            return af1[j][:, i * (CH // 2) + m * P:i * (CH // 2) + (m + 1) * P]

        # ---------------- output projection ----------------
        # oc-pair rounds; ALL FOUR mt partials for the pair stay resident in
        # PSUM (2 [128,1024] A-tiles for mt{0,1} + 4 [128,512] B-tiles for
        # mt{2,3}) across the h0/h1 passes, so the full h0 pass of a round
        # (64 matmuls) covers the AllToAll#1 latency before pass h1 needs it.
        outv = out.rearrange("(mt p) (oc c) -> mt p oc c", p=P, c=CH)
        for ocr in range(2):               # oc pair rounds: {0,1}, {2,3}
            ocs = (2 * ocr, 2 * ocr + 1)
            poA = {oc: psA(f"po{oc}") for oc in ocs}
            poB = {(oc, mt): psB(f"po{oc}{mt}") for oc in ocs for mt in (2, 3)}

            def po_slot(oc, mt):
                if mt < 2:
                    return poA[oc][:, mt * CH:(mt + 1) * CH]
                return poB[(oc, mt)][:]

            for h in range(HPC):
                for mt in range(4):
                    for i in range(NCORES):
                        asl = aslf(h, i, mt)
                        for oc in ocs:
                            nc.tensor.matmul(
                                po_slot(oc, mt), asl,
                                wo_sb[h][oc][:, i * CH:(i + 1) * CH],
                                start=(h == 0 and i == 0),
                                stop=(h == 1 and i == NCORES - 1))
            for oc in ocs:
                for mt in range(4):
                    ot = sb.tile([P, CH], f32, name=f"ot{oc}{mt}", tag="ot",
                                 bufs=2)
                    nc.scalar.copy(out=ot[:], in_=po_slot(oc, mt))
                    nc.sync.dma_start(outv[mt, :, oc], ot[:])

    nc.compile()
    return nc


def _prep_inputs(x, Wq, Wk, Wv, Wo):
    f16 = np.float16
    # x half-chunk groups [B, NC, 2, P, KH*CH]:
    # (b,c,hf,p, k'*CH+ch) = x[b, c*CH+ch, (hf*KH+k')*P+p]
    xt = np.ascontiguousarray(
        x.transpose(0, 2, 1).reshape(B, 2, KH, P, NC, CH)
        .transpose(0, 4, 1, 3, 2, 5).reshape(B, NC, 2, P, KH * CH)).astype(f16)

    def wshard(W, core):
        sl = slice(core * HPC * HD, (core + 1) * HPC * HD)
        return np.ascontiguousarray(
            W[sl].T.reshape(KT, P, HPC * HD).transpose(1, 0, 2)
            .reshape(P, KT * HPC * HD)).astype(f16)

    woh = np.ascontiguousarray(
        Wo.T.reshape(NCORES, HPC, P, NC, CH).transpose(1, 3, 2, 0, 4)
        .reshape(HPC, NC, P, NCORES * CH)).astype(f16)
    in_maps = []
    for core in range(NCORES):
        in_maps.append({
            "xg": xt,
            "wq": wshard(Wq, core),
            "wk": wshard(Wk, core),
            "wv": wshard(Wv, core),
            "wo": woh,
        })
    return in_maps


def kernel(x, rotary_emb, mask, Wq, Wk, Wv, Wo, _trace=False):
    x = np.asarray(x, dtype=np.float32)
    Wq = np.asarray(Wq, dtype=np.float32)
    Wk = np.asarray(Wk, dtype=np.float32)
    Wv = np.asarray(Wv, dtype=np.float32)
    Wo = np.asarray(Wo, dtype=np.float32)

    if "nc" not in _CACHE:
        _CACHE["nc"] = _build()
    nc = _CACHE["nc"]

    from concourse.bass_utils import run_bass_kernel_spmd
    in_maps = _prep_inputs(x, Wq, Wk, Wv, Wo)
    res = run_bass_kernel_spmd(nc, in_maps, core_ids=list(range(NCORES)),
                               trace=_trace)
    _CACHE["last_result"] = res

    flat = np.empty((B * S, D), dtype=np.float32)
    for core in range(NCORES):
        flat[core * MS:(core + 1) * MS, :] = res.results[core]["out"]
    return flat.reshape(B, S, D)
